# revision 16
# baseline (speedup 1.0000x reference)
"""GIN encoder (3-layer, BN, scatter-add message passing) on 8 Trainium2 cores.

Strategy:
  - Nodes sharded contiguously across 8 cores (12500 each); edges owned by dst core.
  - Per layer: per-edge gather of h[src] rows via gpsimd.dma_gather from a
    replicated node-major table in DRAM (4 int16-addressable windows of 2 shards),
    segment-summed into feature-major agg^T tiles via PE matmuls with on-chip
    is_equal selection matrices; MLP runs feature-major with BN folded into
    per-column affines (b1/b2 cancel under BN); BN stats all-reduced across cores;
    next layer's table rebuilt via AllGather of per-core node-major shards.
  - Pooling via per-tile one-hot matmuls into a per-core graph window, combined
    on host; node_rep shards concatenated on host.
"""
import sys
sys.path.insert(0, "/opt/trn_rl_repo")
import math
import os
import numpy as np

import concourse.bass as bass
import concourse.bacc as bacc
import concourse.tile as tile
import concourse.mybir as mybir
from concourse.masks import make_identity

P = 128
F = 128            # feature dim (== P)
NCORES = 8
L = 3
BN_EPS = 1e-5
KT_CALL_CAP = 24   # max K-tiles per dma_gather call


class Cfg:
    def __init__(self, n_nodes=100000, n_edges=1600000, n_graphs=500, rt=4):
        assert n_nodes % NCORES == 0
        self.n_nodes = n_nodes
        self.n_edges = n_edges
        self.n_graphs = n_graphs
        self.shard = n_nodes // NCORES           # nodes per core
        self.nt = math.ceil(self.shard / P)      # dst tiles per core
        self.shard_pad = self.nt * P
        self.rps = self.shard + 1                # table rows per shard (+ zero row)
        self.tbl = self.rps * NCORES
        self.nwin = 4                            # int16 windows (2 shards each)
        self.win_rows = 2 * self.rps
        assert self.win_rows <= 32768
        self.rt = rt                             # dst tiles per gather group
        self.groups = [list(range(g, min(g + rt, self.nt)))
                       for g in range(0, self.nt, rt)]


def _preprocess(cfg, x, src, dst, batch, eps, W1, b1, g1, be1, W2, b2, gO, bO):
    n, sh, nt, nwin, rps = cfg.n_nodes, cfg.shard, cfg.nt, cfg.nwin, cfg.rps
    src = np.asarray(src, dtype=np.int64)
    dst = np.asarray(dst, dtype=np.int64)
    batch = np.asarray(batch, dtype=np.int64)
    x = np.asarray(x, dtype=np.float32)

    r_of = dst // sh
    dl = dst - r_of * sh
    t_of = dl >> 7
    dk = dl & 127
    c_of = src // (2 * sh)
    seg = ((r_of * nt + t_of) * nwin + c_of).astype(np.int64)
    okey = (seg << 7) | dk
    order = np.argsort(okey, kind="stable")
    counts = np.bincount(seg, minlength=NCORES * nt * nwin).reshape(NCORES, nt, nwin)
    seg_starts = np.zeros(NCORES * nt * nwin + 1, dtype=np.int64)
    np.cumsum(counts.reshape(-1), out=seg_starts[1:])

    # uniform K-tile counts across cores
    n_kt = np.ceil(counts.max(axis=0) / P).astype(np.int64)  # [nt, nwin]

    # window-local gather index for each edge's src
    src_r = src // sh
    src_win_local = (src_r - 2 * (src_r // 2)) * rps + (src - src_r * sh)
    assert src_win_local.max() < 2 * rps
    zero_idx = sh  # zero row of first shard in each window

    s_win = (n_kt.sum(axis=0) * P).astype(np.int64)   # slots per window
    kt_win = n_kt.sum(axis=0)                          # K-tiles per window

    idx_src_sorted = src_win_local[order].astype(np.int16)
    dkw_sorted = dk[order].astype(np.int16)

    # gather-call schedule (uniform across cores); per window, calls are
    # consumed in emission order, with "coff" the column offset into the
    # combined per-call stream [idx cols (ktn*8) | dstw cols (ktn)].
    calls = []
    kt_off = [0] * nwin    # running K-tile offset (for slot positions)
    co_off = [0] * nwin    # running column offset in comb arrays
    for grp in cfg.groups:
        gcalls = []
        for c in range(nwin):
            ktl = [(t, j) for t in grp for j in range(int(n_kt[t, c]))]
            ccalls = []
            for a in range(0, len(ktl), KT_CALL_CAP):
                chunk = ktl[a:a + KT_CALL_CAP]
                ccalls.append({"c": c, "kt": chunk, "off": kt_off[c],
                               "coff": co_off[c]})
                kt_off[c] += len(chunk)
                co_off[c] += len(chunk) * 9
            gcalls.append(ccalls)
        calls.append(gcalls)
    for c in range(nwin):
        assert kt_off[c] == kt_win[c]
    comb_cols = list(co_off)

    per_core = []
    for r in range(NCORES):
        streams = []
        for c in range(nwin):
            s_c = int(s_win[c])
            idx_s = np.full(s_c, zero_idx, dtype=np.int16)
            dstw_s = np.zeros(s_c, dtype=np.int16)
            off = 0
            for t in range(nt):
                cnt = int(counts[r, t, c])
                nkt = int(n_kt[t, c])
                if nkt == 0:
                    continue
                s0 = seg_starts[(r * nt + t) * nwin + c]
                idx_s[off:off + cnt] = idx_src_sorted[s0:s0 + cnt]
                dstw_s[off:off + cnt] = dkw_sorted[s0:s0 + cnt]
                off += nkt * P
            assert off == s_c
            # [128, S/16] replicated idx layout; [128, KT] dstw layout
            idx_a = np.tile(idx_s.reshape(-1, 16).T, (8, 1))
            dstw_a = dstw_s.reshape(-1, P).T
            streams.append((idx_a, dstw_a))
        combs = [np.zeros((P, comb_cols[c]), dtype=np.int16)
                 for c in range(nwin)]
        for gcalls in calls:
            for ccalls in gcalls:
                for cl in ccalls:
                    c = cl["c"]
                    ktn = len(cl["kt"])
                    o, co = cl["off"], cl["coff"]
                    idx_a, dstw_a = streams[c]
                    combs[c][:, co:co + ktn * 8] = idx_a[:, o * 8:(o + ktn) * 8]
                    combs[c][:, co + ktn * 8:co + ktn * 9] = dstw_a[:, o:o + ktn]
        per_core.append(combs)

    # graph windows
    gbase = np.array([batch[r * sh] for r in range(NCORES)], dtype=np.int64)
    gid_rel = np.full((NCORES, cfg.shard_pad), -1e6, dtype=np.float32)
    for r in range(NCORES):
        rel = (batch[r * sh:(r + 1) * sh] - gbase[r]).astype(np.float32)
        assert rel.max() < P, "graph span exceeds window"
        gid_rel[r, :sh] = rel
    gid_arrs = [gid_rel[r].reshape(-1, P).T.copy() for r in range(NCORES)]

    # padded node-major table for layer 0
    x_pad = np.zeros((cfg.tbl, F), dtype=np.float32)
    for r in range(NCORES):
        x_pad[r * rps:r * rps + sh] = x[r * sh:(r + 1) * sh]

    # weights
    eps = np.asarray(eps, dtype=np.float32)
    wts = np.zeros((P, 9 * P), dtype=np.float32)
    for l in range(L):
        wts[:, (3 * l + 0) * P:(3 * l + 1) * P] = (1.0 + eps[l]) * W1[l]
        wts[:, (3 * l + 1) * P:(3 * l + 2) * P] = W1[l]
        wts[:, (3 * l + 2) * P:(3 * l + 3) * P] = W2[l]
    bn = np.zeros((P, 4 * L), dtype=np.float32)
    for l in range(L):
        bn[:, 4 * l + 0] = g1[l]
        bn[:, 4 * l + 1] = be1[l]
        bn[:, 4 * l + 2] = gO[l]
        bn[:, 4 * l + 3] = bO[l]
    iota = np.broadcast_to(np.arange(P, dtype=np.float32), (P, P)).copy()

    in_maps = []
    for r in range(NCORES):
        xt = np.zeros((P, cfg.shard_pad), dtype=np.float32)
        xt[:, :sh] = x[r * sh:(r + 1) * sh].T
        m = {"x_pad": x_pad, "wts": wts, "bn": bn, "iota": iota,
             "xT": xt, "gid": gid_arrs[r]}
        for c in range(nwin):
            m[f"comb_w{c}"] = per_core[r][c]
        in_maps.append(m)

    sched = {"n_kt": n_kt, "calls": calls, "comb_cols": comb_cols,
             "kt_win": kt_win}
    host = {"gbase": gbase}
    return in_maps, sched, host


def _build(cfg, sched):
    reps = int(os.environ.get("KREPS", "1"))
    solo = bool(os.environ.get("KSOLO"))
    n_kt, calls = sched["n_kt"], sched["calls"]
    comb_cols = sched["comb_cols"]
    nt, nwin, sh = cfg.nt, cfg.nwin, cfg.shard
    sp, rt = cfg.shard_pad, cfg.rt
    ngrp = len(cfg.groups)
    dt = mybir.dt

    nc = bacc.Bacc("TRN2", target_bir_lowering=False, debug=False,
                   num_devices=1 if solo else NCORES)

    x_pad = nc.dram_tensor("x_pad", [cfg.tbl, F], dt.float32, kind="ExternalInput")
    wts = nc.dram_tensor("wts", [P, 9 * P], dt.float32, kind="ExternalInput")
    bn = nc.dram_tensor("bn", [P, 4 * L], dt.float32, kind="ExternalInput")
    iota_d = nc.dram_tensor("iota", [P, P], dt.float32, kind="ExternalInput")
    xT_d = nc.dram_tensor("xT", [P, sp], dt.float32, kind="ExternalInput")
    gid_d = nc.dram_tensor("gid", [P, nt], dt.float32, kind="ExternalInput")
    comb_d = [nc.dram_tensor(f"comb_w{c}", [P, comb_cols[c]], dt.int16,
                             kind="ExternalInput") for c in range(nwin)]

    node_rep = nc.dram_tensor("node_rep", [sp, L * F], dt.float32,
                              kind="ExternalOutput")
    pooled_o = nc.dram_tensor("pooled", [L, P, F], dt.float32,
                              kind="ExternalOutput")

    with tile.TileContext(nc) as tc:
        with tc.tile_pool(name="big", bufs=1) as bigp, \
             tc.tile_pool(name="gat", bufs=2) as gp, \
             tc.tile_pool(name="mlp", bufs=3) as mp, \
             tc.tile_pool(name="cst", bufs=1) as cp, \
             tc.tile_pool(name="ps", bufs=2, space="PSUM") as pp, \
             tc.tile_pool(name="psp", bufs=1, space="PSUM") as ppool, \
             tc.tile_pool(name="dram", bufs=2, space="DRAM") as dr:

            wts_t = cp.tile([P, 9 * P], dt.float32, tag="wts")
            nc.sync.dma_start(out=wts_t[:], in_=wts[:])
            bn_t = cp.tile([P, 4 * L], dt.float32, tag="bn")
            nc.sync.dma_start(out=bn_t[:], in_=bn[:])
            iota_t = cp.tile([P, P], dt.float32, tag="iota")
            nc.sync.dma_start(out=iota_t[:], in_=iota_d[:])
            gid_t = cp.tile([P, nt], dt.float32, tag="gid")
            nc.sync.dma_start(out=gid_t[:], in_=gid_d[:])
            ident = cp.tile([P, P], dt.float32, tag="ident")
            make_identity(nc, ident[:])
            zrow = cp.tile([1, F], dt.float32, tag="zrow")
            nc.vector.memset(zrow[:], 0.0)

            hT = bigp.tile([P, sp], dt.float32, tag="hT")
            zT = bigp.tile([P, sp], dt.float32, tag="zT")

            for _rep in range(reps):
              nc.sync.dma_start(out=hT[:], in_=xT_d[:])
              tables = [x_pad.ap()]
              for l in range(L):
                tbl_ap = tables[l]
                w1a = wts_t[:, (3 * l + 0) * P:(3 * l + 1) * P]
                w1 = wts_t[:, (3 * l + 1) * P:(3 * l + 2) * P]
                w2 = wts_t[:, (3 * l + 2) * P:(3 * l + 3) * P]

                sumc = mp.tile([P, 2 * ngrp], dt.float32, tag="sumc")
                # ---- aggregation + first linear ----
                for gi, grp in enumerate(cfg.groups):
                    gw = len(grp) * P
                    agg_ps = pp.tile([P, rt * P], dt.float32, tag="aggps")
                    nmm = sum(int(n_kt[t, :].sum()) for t in grp)
                    mmi = 0
                    for c in range(nwin):
                        for cl in calls[gi][c]:
                            ktn = len(cl["kt"])
                            slots = ktn * P
                            co = cl["coff"]
                            comb_t = gp.tile([P, ktn * 9], dt.int16, tag="comb")
                            nc.scalar.dma_start(
                                out=comb_t[:],
                                in_=comb_d[c][:, co:co + ktn * 9])
                            g_t = gp.tile([P, ktn, F], dt.float32, tag="G")
                            nc.gpsimd.dma_gather(
                                out_ap=g_t[:],
                                in_ap=tbl_ap[c * cfg.win_rows:(c + 1) * cfg.win_rows, :],
                                idxs_ap=comb_t[:, :ktn * 8],
                                num_idxs=slots,
                                num_idxs_reg=slots,
                                elem_size=F,
                                single_packet=False,
                            )
                            dstw_f = gp.tile([P, ktn], dt.float32, tag="dstwf")
                            nc.vector.tensor_copy(
                                out=dstw_f[:], in_=comb_t[:, ktn * 8:ktn * 9])
                            sel_t = gp.tile([P, ktn, P], dt.float32, tag="sel")
                            nc.vector.tensor_tensor(
                                out=sel_t[:],
                                in0=dstw_f[:].to_broadcast([P, ktn, P]),
                                in1=iota_t[:].rearrange("p (k e) -> p k e", k=1)
                                             .to_broadcast([P, ktn, P]),
                                op=mybir.AluOpType.is_equal)
                            for kk, (t, j) in enumerate(cl["kt"]):
                                ti = t - grp[0]
                                nc.tensor.matmul(
                                    out=agg_ps[:, ti * P:(ti + 1) * P],
                                    lhsT=g_t[:, kk, :],
                                    rhs=sel_t[:, kk, :],
                                    start=(mmi == 0),
                                    stop=(mmi == nmm - 1),
                                    skip_group_check=True)
                                mmi += 1
                    cols = slice(grp[0] * P, grp[0] * P + gw)
                    agg_sb = mp.tile([P, rt * P], dt.float32, tag="aggsb")
                    nc.vector.tensor_copy(out=agg_sb[:, :gw], in_=agg_ps[:, :gw])
                    for t in grp:
                        if int(n_kt[t, :].sum()) == 0:
                            ti = t - grp[0]
                            nc.vector.memset(agg_sb[:, ti * P:(ti + 1) * P], 0.0)
                    z_ps = pp.tile([P, rt * P], dt.float32, tag="zps")
                    nc.tensor.matmul(out=z_ps[:, :gw], lhsT=w1a, rhs=hT[:, cols],
                                     start=True, stop=False, skip_group_check=True)
                    nc.tensor.matmul(out=z_ps[:, :gw], lhsT=w1, rhs=agg_sb[:, :gw],
                                     start=False, stop=True, skip_group_check=True)
                    sq_s = mp.tile([P, rt * P], dt.float32, tag="sqs")
                    nc.vector.tensor_scalar(
                        out=zT[:, cols], in0=z_ps[:, :gw], scalar1=1.0, scalar2=0.0,
                        op0=mybir.AluOpType.mult, op1=mybir.AluOpType.add,
                        accum_out=sumc[:, gi:gi + 1])
                    nc.scalar.activation(
                        out=sq_s[:, :gw], in_=zT[:, cols],
                        func=mybir.ActivationFunctionType.Square,
                        accum_out=sumc[:, ngrp + gi:ngrp + gi + 1])

                # ---- BN1 ----
                s1, d1 = _stats(nc, mp, dr, cfg, solo, sumc, ngrp,
                                bn_t[:, 4 * l:4 * l + 1],
                                bn_t[:, 4 * l + 1:4 * l + 2])

                # ---- t = relu(s1*z+d1); z2 = W2^T t; stats2 ----
                sumc2 = mp.tile([P, 2 * ngrp], dt.float32, tag="sumc2")
                for gi, grp in enumerate(cfg.groups):
                    gw = len(grp) * P
                    cols = slice(grp[0] * P, grp[0] * P + gw)
                    t_s = mp.tile([P, rt * P], dt.float32, tag="ts")
                    nc.scalar.activation(
                        out=t_s[:, :gw], in_=zT[:, cols],
                        func=mybir.ActivationFunctionType.Relu,
                        bias=d1[:, 0:1], scale=s1[:, 0:1])
                    if grp[-1] == nt - 1 and sp > sh:
                        po = sh - grp[0] * P
                        nc.vector.memset(t_s[:, po:gw], 0.0)
                    z2_ps = pp.tile([P, rt * P], dt.float32, tag="zps")
                    nc.tensor.matmul(out=z2_ps[:, :gw], lhsT=w2, rhs=t_s[:, :gw],
                                     start=True, stop=True, skip_group_check=True)
                    sq_s = mp.tile([P, rt * P], dt.float32, tag="sqs")
                    nc.vector.tensor_scalar(
                        out=zT[:, cols], in0=z2_ps[:, :gw], scalar1=1.0,
                        scalar2=0.0, op0=mybir.AluOpType.mult,
                        op1=mybir.AluOpType.add,
                        accum_out=sumc2[:, gi:gi + 1])
                    nc.scalar.activation(
                        out=sq_s[:, :gw], in_=zT[:, cols],
                        func=mybir.ActivationFunctionType.Square,
                        accum_out=sumc2[:, ngrp + gi:ngrp + gi + 1])

                s2, d2 = _stats(nc, mp, dr, cfg, solo, sumc2, ngrp,
                                bn_t[:, 4 * l + 2:4 * l + 3],
                                bn_t[:, 4 * l + 3:4 * l + 4])

                # ---- h3 = relu(s2*z2+d2) -> hT; epilogue ----
                if l < L - 1:
                    ag_in = dr.tile([cfg.rps, F], dt.float32, tag="agin")
                    ag_out = dr.tile([cfg.tbl, F], dt.float32, tag="agout")
                    nc.sync.dma_start(out=ag_in[sh:sh + 1, :], in_=zrow[:])
                pooled_ps = ppool.tile([P, F], dt.float32, tag="poolps")
                for gi, grp in enumerate(cfg.groups):
                    gw = len(grp) * P
                    cols = slice(grp[0] * P, grp[0] * P + gw)
                    nc.scalar.activation(
                        out=hT[:, cols], in_=zT[:, cols],
                        func=mybir.ActivationFunctionType.Relu,
                        bias=d2[:, 0:1], scale=s2[:, 0:1])
                    if grp[-1] == nt - 1 and sp > sh:
                        nc.vector.memset(hT[:, sh:sp], 0.0)
                    stage = mp.tile([P, rt, P], dt.float32, tag="stage")
                    for t in grp:
                        ti = t - grp[0]
                        nm_ps = pp.tile([P, P], dt.float32, tag="nmps")
                        nc.tensor.transpose(
                            out=nm_ps[:], in_=hT[:, t * P:(t + 1) * P],
                            identity=ident[:])
                        nc.vector.tensor_copy(out=stage[:, ti, :], in_=nm_ps[:])
                        oh = mp.tile([P, P], dt.float32, tag="oh")
                        nc.vector.tensor_tensor(
                            out=oh[:], in0=gid_t[:, t:t + 1].to_broadcast([P, P]),
                            in1=iota_t[:], op=mybir.AluOpType.is_equal)
                        nc.tensor.matmul(
                            out=pooled_ps[:], lhsT=oh[:], rhs=stage[:, ti, :],
                            start=(t == 0), stop=(t == nt - 1),
                            skip_group_check=True)
                    jn = len(grp)
                    r0 = grp[0] * P
                    nc.sync.dma_start(
                        out=node_rep[r0:r0 + jn * P, l * F:(l + 1) * F]
                            .rearrange("(j p) f -> p j f", p=P),
                        in_=stage[:, :jn, :])
                    if l < L - 1:
                        rows = min(jn * P, sh - r0)
                        jf = rows // P
                        if jf > 0:
                            nc.sync.dma_start(
                                out=ag_in[r0:r0 + jf * P, :]
                                    .rearrange("(j p) f -> p j f", p=P),
                                in_=stage[:, :jf, :])
                        rem = rows - jf * P
                        if rem > 0:
                            nc.sync.dma_start(
                                out=ag_in[r0 + jf * P:r0 + jf * P + rem, :],
                                in_=stage[:rem, jf, :])
                pool_sb = mp.tile([P, F], dt.float32, tag="poolsb")
                nc.vector.tensor_copy(out=pool_sb[:], in_=pooled_ps[:])
                nc.sync.dma_start(out=pooled_o[l, :, :], in_=pool_sb[:])

                if l < L - 1:
                    if solo:
                        tables.append(x_pad.ap())
                    else:
                        nc.gpsimd.collective_compute(
                            "AllGather", mybir.AluOpType.bypass,
                            replica_groups=[list(range(NCORES))],
                            ins=[ag_in.opt()], outs=[ag_out.opt()])
                        tables.append(ag_out[:])

    nc.compile()
    return nc


def _stats(nc, mp, dr, cfg, solo, sumc, ngrp, gamma, beta):
    """Reduce per-group (sum, sumsq), AllReduce, return s=[g*rsqrt(var+eps)],
    d=[beta - mean*s] as [P,1] APs."""
    dt = mybir.dt
    st = mp.tile([P, 2], dt.float32, tag="st")
    nc.vector.tensor_reduce(
        out=st[:, 0:1], in_=sumc[:, 0:ngrp],
        axis=mybir.AxisListType.X, op=mybir.AluOpType.add)
    nc.vector.tensor_reduce(
        out=st[:, 1:2], in_=sumc[:, ngrp:2 * ngrp],
        axis=mybir.AxisListType.X, op=mybir.AluOpType.add)
    ar_i = dr.tile([P, 2], dt.float32, tag="ari")
    ar_o = dr.tile([P, 2], dt.float32, tag="aro")
    nc.gpsimd.dma_start(out=ar_i[:], in_=st[:])
    if solo:
        nc.gpsimd.dma_start(out=ar_o[:], in_=ar_i[:])
    else:
        nc.gpsimd.collective_compute(
            "AllReduce", mybir.AluOpType.add,
            replica_groups=[list(range(NCORES))],
            ins=[ar_i.opt()], outs=[ar_o.opt()])
    stg = mp.tile([P, 2], dt.float32, tag="stg")
    nc.sync.dma_start(out=stg[:], in_=ar_o[:])
    inv_n = 1.0 / cfg.n_nodes
    w = mp.tile([P, 6], dt.float32, tag="statw")
    # w: 0=mean 1=E2 2=var 3=recip->rsqrt 4=s 5=d
    nc.vector.tensor_scalar(out=w[:, 0:1], in0=stg[:, 0:1], scalar1=inv_n,
                            scalar2=None, op0=mybir.AluOpType.mult)
    nc.vector.tensor_scalar(out=w[:, 1:2], in0=stg[:, 1:2], scalar1=inv_n,
                            scalar2=None, op0=mybir.AluOpType.mult)
    nc.vector.tensor_tensor(out=w[:, 2:3], in0=w[:, 0:1], in1=w[:, 0:1],
                            op=mybir.AluOpType.mult)
    nc.vector.tensor_tensor(out=w[:, 2:3], in0=w[:, 1:2], in1=w[:, 2:3],
                            op=mybir.AluOpType.subtract)
    nc.vector.tensor_scalar(out=w[:, 2:3], in0=w[:, 2:3], scalar1=BN_EPS,
                            scalar2=None, op0=mybir.AluOpType.add)
    nc.vector.reciprocal(out=w[:, 3:4], in_=w[:, 2:3])
    nc.scalar.activation(out=w[:, 3:4], in_=w[:, 3:4],
                         func=mybir.ActivationFunctionType.Sqrt)
    nc.vector.tensor_tensor(out=w[:, 4:5], in0=gamma, in1=w[:, 3:4],
                            op=mybir.AluOpType.mult)
    nc.vector.tensor_tensor(out=w[:, 5:6], in0=w[:, 0:1], in1=w[:, 4:5],
                            op=mybir.AluOpType.mult)
    nc.vector.tensor_tensor(out=w[:, 5:6], in0=beta, in1=w[:, 5:6],
                            op=mybir.AluOpType.subtract)
    return w[:, 4:5], w[:, 5:6]


_CACHE = {}


def _get_nc(cfg, sched, key):
    if key not in _CACHE:
        _CACHE[key] = _build(cfg, sched)
    return _CACHE[key]


class _Exec:
    """PJRT executor mirroring bass2jax.run_bass_via_pjrt, with a cached
    jitted callable so executions can be repeated/timed."""

    def __init__(self, nc):
        import jax
        from jax.sharding import Mesh, PartitionSpec
        from jax.experimental.shard_map import shard_map
        from concourse import bass2jax, mybir as mb
        bass2jax.install_neuronx_cc_hook()
        self.jax = jax
        partition_name = (nc.partition_id_tensor.name
                          if nc.partition_id_tensor else None)
        in_names, out_names, out_avals, zero_outs = [], [], [], []
        for alloc in nc.m.functions[0].allocations:
            if not isinstance(alloc, mb.MemoryLocationSet):
                continue
            name = alloc.memorylocations[0].name
            if alloc.kind == "ExternalInput":
                if name != partition_name:
                    in_names.append(name)
            elif alloc.kind == "ExternalOutput":
                out_names.append(name)
                shape = tuple(alloc.tensor_shape)
                dtp = mb.dt.np(alloc.dtype)
                out_avals.append(jax.core.ShapedArray(shape, dtp))
                zero_outs.append(np.zeros(shape, dtp))
        self.in_names = list(in_names)
        self.out_names = out_names
        self.out_avals = out_avals
        self.zero_outs = zero_outs
        n_params, n_outs = len(in_names), len(out_avals)
        self.n_params = n_params
        all_in = in_names + out_names + ([partition_name] if partition_name else [])
        donate = tuple(range(n_params, n_params + n_outs))

        def _body(*args):
            operands = list(args)
            if partition_name is not None:
                operands.append(bass2jax.partition_id_tensor())
            outs = bass2jax._bass_exec_p.bind(
                *operands, out_avals=tuple(out_avals), in_names=tuple(all_in),
                out_names=tuple(out_names), lowering_input_output_aliases=(),
                sim_require_finite=True, sim_require_nnan=True, nc=nc)
            return tuple(outs)

        devices = jax.devices()[:NCORES]
        self.mesh = Mesh(np.asarray(devices), ("core",))
        self.spec = PartitionSpec("core")
        in_specs = (self.spec,) * (n_params + n_outs)
        out_specs = (self.spec,) * n_outs
        self.fn = jax.jit(
            shard_map(_body, mesh=self.mesh, in_specs=in_specs,
                      out_specs=out_specs, check_rep=False),
            donate_argnums=donate, keep_unused=True)
        self.dev_in = None

    def put_inputs(self, in_maps):
        import jax
        from jax.sharding import NamedSharding
        sh = NamedSharding(self.mesh, self.spec)
        concat = [np.concatenate([np.asarray(m[n]) for m in in_maps], axis=0)
                  for n in self.in_names]
        self.dev_in = [jax.device_put(a, sh) for a in concat]

    def _dev_zeros(self):
        import jax
        from jax.sharding import NamedSharding
        sh = NamedSharding(self.mesh, self.spec)
        return [jax.device_put(
            np.zeros((NCORES * z.shape[0], *z.shape[1:]), z.dtype), sh)
            for z in self.zero_outs]

    def execute(self):
        out = self.fn(*self.dev_in, *self._dev_zeros())
        self.jax.block_until_ready(out)
        return out

    def results(self, out):
        res = []
        for c in range(NCORES):
            res.append({
                name: np.asarray(out[i]).reshape(
                    NCORES, *self.out_avals[i].shape)[c]
                for i, name in enumerate(self.out_names)})
        return res

    def time(self, iters=5):
        import time
        ts = []
        for _ in range(iters):
            zeros = self._dev_zeros()
            self.jax.block_until_ready(zeros)
            t0 = time.perf_counter()
            out = self.fn(*self.dev_in, *zeros)
            self.jax.block_until_ready(out)
            ts.append(time.perf_counter() - t0)
        return ts


def _assemble(cfg, host, results):
    sh, n_g = cfg.shard, cfg.n_graphs
    node_rep = np.concatenate(
        [results[r]["node_rep"][:sh] for r in range(NCORES)], axis=0)
    graph_rep = np.zeros((n_g, L * F), dtype=np.float32)
    for r in range(NCORES):
        gb = int(host["gbase"][r])
        w = min(P, n_g - gb)
        blk = results[r]["pooled"]
        for l in range(L):
            graph_rep[gb:gb + w, l * F:(l + 1) * F] += blk[l][:w]
    return graph_rep, node_rep


def get_exec(cfg, inputs):
    in_maps, sched, host = _preprocess(cfg, **inputs)
    reps = os.environ.get("KREPS", "1")
    nc = _get_nc(cfg, sched, (cfg.n_nodes, cfg.n_edges, cfg.n_graphs, reps))
    key = ("exec", cfg.n_nodes, cfg.n_edges, cfg.n_graphs, reps)
    if key not in _CACHE:
        _CACHE[key] = _Exec(nc)
    ex = _CACHE[key]
    ex.put_inputs(in_maps)
    return ex, host


def run(cfg, inputs, trace=False):
    ex, host = get_exec(cfg, inputs)
    results = ex.results(ex.execute())
    graph_rep, node_rep = _assemble(cfg, host, results)

    class R:
        exec_time_ns = None
        instructions_and_trace = None
    r = R()
    r.results = results
    return (graph_rep, node_rep), r


def kernel(**inputs):
    cfg = Cfg()
    (graph_rep, node_rep), _ = run(cfg, inputs)
    return (graph_rep, node_rep)


# revision 17
# speedup vs baseline: 1.1990x; 1.1990x over previous
"""GIN encoder (3-layer, BN, scatter-add message passing) on 8 Trainium2 cores.

Strategy:
  - Nodes sharded contiguously across 8 cores (12500 each); edges owned by dst core.
  - Per layer: per-edge gather of h[src] rows via gpsimd.dma_gather from a
    replicated node-major table in DRAM (4 int16-addressable windows of 2 shards),
    segment-summed into feature-major agg^T tiles via PE matmuls with on-chip
    is_equal selection matrices; MLP runs feature-major with BN folded into
    per-column affines (b1/b2 cancel under BN); BN stats all-reduced across cores;
    next layer's table rebuilt via AllGather of per-core node-major shards.
  - Pooling via per-tile one-hot matmuls into a per-core graph window, combined
    on host; node_rep shards concatenated on host.
"""
import sys
sys.path.insert(0, "/opt/trn_rl_repo")
import math
import os
import numpy as np

import concourse.bass as bass
import concourse.bacc as bacc
import concourse.tile as tile
import concourse.mybir as mybir
from concourse.masks import make_identity

P = 128
F = 128            # feature dim (== P)
NCORES = 8
L = 3
BN_EPS = 1e-5
KT_CALL_CAP = 24   # max K-tiles per dma_gather call


class Cfg:
    def __init__(self, n_nodes=100000, n_edges=1600000, n_graphs=500, rt=4):
        assert n_nodes % NCORES == 0
        self.n_nodes = n_nodes
        self.n_edges = n_edges
        self.n_graphs = n_graphs
        self.shard = n_nodes // NCORES           # nodes per core
        self.nt = math.ceil(self.shard / P)      # dst tiles per core
        self.shard_pad = self.nt * P
        self.rps = self.shard + 1                # table rows per shard (+ zero row)
        self.tbl = self.rps * NCORES
        self.nwin = 4                            # int16 windows (2 shards each)
        self.win_rows = 2 * self.rps
        assert self.win_rows <= 32768
        self.rt = rt                             # dst tiles per gather group
        self.groups = [list(range(g, min(g + rt, self.nt)))
                       for g in range(0, self.nt, rt)]


def _preprocess(cfg, x, src, dst, batch, eps, W1, b1, g1, be1, W2, b2, gO, bO):
    n, sh, nt, nwin, rps = cfg.n_nodes, cfg.shard, cfg.nt, cfg.nwin, cfg.rps
    src = np.asarray(src, dtype=np.int64)
    dst = np.asarray(dst, dtype=np.int64)
    batch = np.asarray(batch, dtype=np.int64)
    x = np.asarray(x, dtype=np.float32)

    r_of = dst // sh
    dl = dst - r_of * sh
    t_of = dl >> 7
    dk = dl & 127
    c_of = src // (2 * sh)
    seg = ((r_of * nt + t_of) * nwin + c_of).astype(np.int64)
    okey = (seg << 7) | dk
    order = np.argsort(okey, kind="stable")
    counts = np.bincount(seg, minlength=NCORES * nt * nwin).reshape(NCORES, nt, nwin)
    seg_starts = np.zeros(NCORES * nt * nwin + 1, dtype=np.int64)
    np.cumsum(counts.reshape(-1), out=seg_starts[1:])

    # uniform K-tile counts across cores
    n_kt = np.ceil(counts.max(axis=0) / P).astype(np.int64)  # [nt, nwin]

    # window-local gather index for each edge's src
    src_r = src // sh
    src_win_local = (src_r - 2 * (src_r // 2)) * rps + (src - src_r * sh)
    assert src_win_local.max() < 2 * rps
    zero_idx = sh  # zero row of first shard in each window

    s_win = (n_kt.sum(axis=0) * P).astype(np.int64)   # slots per window
    kt_win = n_kt.sum(axis=0)                          # K-tiles per window

    idx_src_sorted = src_win_local[order].astype(np.int16)
    dkw_sorted = dk[order].astype(np.int16)

    # gather-call schedule (uniform across cores); per window, calls are
    # consumed in emission order, with "coff" the column offset into the
    # combined per-call stream [idx cols (ktn*8) | dstw cols (ktn)].
    calls = []
    kt_off = [0] * nwin    # running K-tile offset (for slot positions)
    co_off = [0] * nwin    # running column offset in comb arrays
    for grp in cfg.groups:
        gcalls = []
        for c in range(nwin):
            ktl = [(t, j) for t in grp for j in range(int(n_kt[t, c]))]
            ccalls = []
            for a in range(0, len(ktl), KT_CALL_CAP):
                chunk = ktl[a:a + KT_CALL_CAP]
                ccalls.append({"c": c, "kt": chunk, "off": kt_off[c],
                               "coff": co_off[c]})
                kt_off[c] += len(chunk)
                co_off[c] += len(chunk) * 9
            gcalls.append(ccalls)
        calls.append(gcalls)
    for c in range(nwin):
        assert kt_off[c] == kt_win[c]
    comb_cols = list(co_off)

    per_core = []
    for r in range(NCORES):
        streams = []
        for c in range(nwin):
            s_c = int(s_win[c])
            idx_s = np.full(s_c, zero_idx, dtype=np.int16)
            dstw_s = np.zeros(s_c, dtype=np.int16)
            off = 0
            for t in range(nt):
                cnt = int(counts[r, t, c])
                nkt = int(n_kt[t, c])
                if nkt == 0:
                    continue
                s0 = seg_starts[(r * nt + t) * nwin + c]
                idx_s[off:off + cnt] = idx_src_sorted[s0:s0 + cnt]
                dstw_s[off:off + cnt] = dkw_sorted[s0:s0 + cnt]
                off += nkt * P
            assert off == s_c
            # [128, S/16] replicated idx layout; [128, KT] dstw layout
            idx_a = np.tile(idx_s.reshape(-1, 16).T, (8, 1))
            dstw_a = dstw_s.reshape(-1, P).T
            streams.append((idx_a, dstw_a))
        combs = [np.zeros((P, comb_cols[c]), dtype=np.int16)
                 for c in range(nwin)]
        for gcalls in calls:
            for ccalls in gcalls:
                for cl in ccalls:
                    c = cl["c"]
                    ktn = len(cl["kt"])
                    o, co = cl["off"], cl["coff"]
                    idx_a, dstw_a = streams[c]
                    combs[c][:, co:co + ktn * 8] = idx_a[:, o * 8:(o + ktn) * 8]
                    combs[c][:, co + ktn * 8:co + ktn * 9] = dstw_a[:, o:o + ktn]
        per_core.append(combs)

    # graph windows
    gbase = np.array([batch[r * sh] for r in range(NCORES)], dtype=np.int64)
    gid_rel = np.full((NCORES, cfg.shard_pad), -1e6, dtype=np.float32)
    for r in range(NCORES):
        rel = (batch[r * sh:(r + 1) * sh] - gbase[r]).astype(np.float32)
        assert rel.max() < P, "graph span exceeds window"
        gid_rel[r, :sh] = rel
    gid_arrs = [gid_rel[r].reshape(-1, P).T.copy() for r in range(NCORES)]

    # padded node-major table for layer 0
    x_pad = np.zeros((cfg.tbl, F), dtype=np.float32)
    for r in range(NCORES):
        x_pad[r * rps:r * rps + sh] = x[r * sh:(r + 1) * sh]

    # weights
    eps = np.asarray(eps, dtype=np.float32)
    wts = np.zeros((P, 9 * P), dtype=np.float32)
    for l in range(L):
        wts[:, (3 * l + 0) * P:(3 * l + 1) * P] = (1.0 + eps[l]) * W1[l]
        wts[:, (3 * l + 1) * P:(3 * l + 2) * P] = W1[l]
        wts[:, (3 * l + 2) * P:(3 * l + 3) * P] = W2[l]
    bn = np.zeros((P, 4 * L), dtype=np.float32)
    for l in range(L):
        bn[:, 4 * l + 0] = g1[l]
        bn[:, 4 * l + 1] = be1[l]
        bn[:, 4 * l + 2] = gO[l]
        bn[:, 4 * l + 3] = bO[l]
    iota = np.broadcast_to(np.arange(P, dtype=np.float32), (P, P)).copy()

    in_maps = []
    for r in range(NCORES):
        xt = np.zeros((P, cfg.shard_pad), dtype=np.float32)
        xt[:, :sh] = x[r * sh:(r + 1) * sh].T
        m = {"x_pad": x_pad, "wts": wts, "bn": bn, "iota": iota,
             "xT": xt, "gid": gid_arrs[r]}
        for c in range(nwin):
            m[f"comb_w{c}"] = per_core[r][c]
        in_maps.append(m)

    sched = {"n_kt": n_kt, "calls": calls, "comb_cols": comb_cols,
             "kt_win": kt_win}
    host = {"gbase": gbase}
    return in_maps, sched, host


def _build(cfg, sched):
    reps = int(os.environ.get("KREPS", "1"))
    solo = bool(os.environ.get("KSOLO"))
    nocoll = bool(os.environ.get("KNOCOLL")) or solo
    n_kt, calls = sched["n_kt"], sched["calls"]
    comb_cols = sched["comb_cols"]
    nt, nwin, sh = cfg.nt, cfg.nwin, cfg.shard
    sp, rt = cfg.shard_pad, cfg.rt
    ngrp = len(cfg.groups)
    dt = mybir.dt

    nc = bacc.Bacc("TRN2", target_bir_lowering=False, debug=False,
                   num_devices=1 if solo else NCORES)

    x_pad = nc.dram_tensor("x_pad", [cfg.tbl, F], dt.float32, kind="ExternalInput")
    wts = nc.dram_tensor("wts", [P, 9 * P], dt.float32, kind="ExternalInput")
    bn = nc.dram_tensor("bn", [P, 4 * L], dt.float32, kind="ExternalInput")
    iota_d = nc.dram_tensor("iota", [P, P], dt.float32, kind="ExternalInput")
    xT_d = nc.dram_tensor("xT", [P, sp], dt.float32, kind="ExternalInput")
    gid_d = nc.dram_tensor("gid", [P, nt], dt.float32, kind="ExternalInput")
    comb_d = [nc.dram_tensor(f"comb_w{c}", [P, comb_cols[c]], dt.int16,
                             kind="ExternalInput") for c in range(nwin)]

    node_rep = nc.dram_tensor("node_rep", [sp, L * F], dt.float32,
                              kind="ExternalOutput")
    pooled_o = nc.dram_tensor("pooled", [L, P, F], dt.float32,
                              kind="ExternalOutput")

    with tile.TileContext(nc) as tc:
        with tc.tile_pool(name="big", bufs=1) as bigp, \
             tc.tile_pool(name="gat", bufs=2) as gp, \
             tc.tile_pool(name="mlp", bufs=3) as mp, \
             tc.tile_pool(name="cst", bufs=1) as cp, \
             tc.tile_pool(name="ps", bufs=2, space="PSUM") as pp, \
             tc.tile_pool(name="psp", bufs=1, space="PSUM") as ppool, \
             tc.tile_pool(name="dram", bufs=2, space="DRAM") as dr:

            wts_t = cp.tile([P, 9 * P], dt.float32, tag="wts")
            nc.sync.dma_start(out=wts_t[:], in_=wts[:])
            bn_t = cp.tile([P, 4 * L], dt.float32, tag="bn")
            nc.sync.dma_start(out=bn_t[:], in_=bn[:])
            iota_t = cp.tile([P, P], dt.float32, tag="iota")
            nc.sync.dma_start(out=iota_t[:], in_=iota_d[:])
            gid_t = cp.tile([P, nt], dt.float32, tag="gid")
            nc.sync.dma_start(out=gid_t[:], in_=gid_d[:])
            ident = cp.tile([P, P], dt.float32, tag="ident")
            make_identity(nc, ident[:])
            zrow = cp.tile([1, F], dt.float32, tag="zrow")
            nc.vector.memset(zrow[:], 0.0)

            hT = bigp.tile([P, sp], dt.float32, tag="hT")
            zT = bigp.tile([P, sp], dt.float32, tag="zT")

            for _rep in range(reps):
              nc.sync.dma_start(out=hT[:], in_=xT_d[:])
              tables = [x_pad.ap()]
              for l in range(L):
                tbl_ap = tables[l]
                w1a = wts_t[:, (3 * l + 0) * P:(3 * l + 1) * P]
                w1 = wts_t[:, (3 * l + 1) * P:(3 * l + 2) * P]
                w2 = wts_t[:, (3 * l + 2) * P:(3 * l + 3) * P]

                sumc = mp.tile([P, 2 * ngrp], dt.float32, tag="sumc")
                # ---- aggregation + first linear ----
                for gi, grp in enumerate(cfg.groups):
                    gw = len(grp) * P
                    agg_ps = pp.tile([P, rt * P], dt.float32, tag="aggps")
                    nmm = sum(int(n_kt[t, :].sum()) for t in grp)
                    mmi = 0
                    for c in range(nwin):
                        for cl in calls[gi][c]:
                            ktn = len(cl["kt"])
                            slots = ktn * P
                            co = cl["coff"]
                            comb_t = gp.tile([P, ktn * 9], dt.int16, tag="comb")
                            nc.scalar.dma_start(
                                out=comb_t[:],
                                in_=comb_d[c][:, co:co + ktn * 9])
                            g_t = gp.tile([P, ktn, F], dt.float32, tag="G")
                            nc.gpsimd.dma_gather(
                                out_ap=g_t[:],
                                in_ap=tbl_ap[c * cfg.win_rows:(c + 1) * cfg.win_rows, :],
                                idxs_ap=comb_t[:, :ktn * 8],
                                num_idxs=slots,
                                num_idxs_reg=slots,
                                elem_size=F,
                                single_packet=False,
                            )
                            dstw_f = gp.tile([P, ktn], dt.float32, tag="dstwf")
                            nc.vector.tensor_copy(
                                out=dstw_f[:], in_=comb_t[:, ktn * 8:ktn * 9])
                            sel_t = gp.tile([P, ktn, P], dt.float32, tag="sel")
                            nc.vector.tensor_tensor(
                                out=sel_t[:],
                                in0=dstw_f[:].to_broadcast([P, ktn, P]),
                                in1=iota_t[:].rearrange("p (k e) -> p k e", k=1)
                                             .to_broadcast([P, ktn, P]),
                                op=mybir.AluOpType.is_equal)
                            for kk, (t, j) in enumerate(cl["kt"]):
                                ti = t - grp[0]
                                nc.tensor.matmul(
                                    out=agg_ps[:, ti * P:(ti + 1) * P],
                                    lhsT=g_t[:, kk, :],
                                    rhs=sel_t[:, kk, :],
                                    start=(mmi == 0),
                                    stop=(mmi == nmm - 1),
                                    skip_group_check=True)
                                mmi += 1
                    cols = slice(grp[0] * P, grp[0] * P + gw)
                    agg_sb = mp.tile([P, rt * P], dt.float32, tag="aggsb")
                    nc.vector.tensor_copy(out=agg_sb[:, :gw], in_=agg_ps[:, :gw])
                    for t in grp:
                        if int(n_kt[t, :].sum()) == 0:
                            ti = t - grp[0]
                            nc.vector.memset(agg_sb[:, ti * P:(ti + 1) * P], 0.0)
                    z_ps = pp.tile([P, rt * P], dt.float32, tag="zps")
                    nc.tensor.matmul(out=z_ps[:, :gw], lhsT=w1a, rhs=hT[:, cols],
                                     start=True, stop=False, skip_group_check=True)
                    nc.tensor.matmul(out=z_ps[:, :gw], lhsT=w1, rhs=agg_sb[:, :gw],
                                     start=False, stop=True, skip_group_check=True)
                    sq_s = mp.tile([P, rt * P], dt.float32, tag="sqs")
                    nc.vector.tensor_scalar(
                        out=zT[:, cols], in0=z_ps[:, :gw], scalar1=1.0, scalar2=0.0,
                        op0=mybir.AluOpType.mult, op1=mybir.AluOpType.add,
                        accum_out=sumc[:, gi:gi + 1])
                    nc.scalar.activation(
                        out=sq_s[:, :gw], in_=zT[:, cols],
                        func=mybir.ActivationFunctionType.Square,
                        accum_out=sumc[:, ngrp + gi:ngrp + gi + 1])

                # ---- BN1 ----
                s1, d1 = _stats(nc, mp, dr, cfg, solo, sumc, ngrp,
                                bn_t[:, 4 * l:4 * l + 1],
                                bn_t[:, 4 * l + 1:4 * l + 2])

                # ---- t = relu(s1*z+d1); z2 = W2^T t; stats2 ----
                sumc2 = mp.tile([P, 2 * ngrp], dt.float32, tag="sumc2")
                for gi, grp in enumerate(cfg.groups):
                    gw = len(grp) * P
                    cols = slice(grp[0] * P, grp[0] * P + gw)
                    t_s = mp.tile([P, rt * P], dt.float32, tag="ts")
                    nc.scalar.activation(
                        out=t_s[:, :gw], in_=zT[:, cols],
                        func=mybir.ActivationFunctionType.Relu,
                        bias=d1[:, 0:1], scale=s1[:, 0:1])
                    if grp[-1] == nt - 1 and sp > sh:
                        po = sh - grp[0] * P
                        nc.vector.memset(t_s[:, po:gw], 0.0)
                    z2_ps = pp.tile([P, rt * P], dt.float32, tag="zps")
                    nc.tensor.matmul(out=z2_ps[:, :gw], lhsT=w2, rhs=t_s[:, :gw],
                                     start=True, stop=True, skip_group_check=True)
                    sq_s = mp.tile([P, rt * P], dt.float32, tag="sqs")
                    nc.vector.tensor_scalar(
                        out=zT[:, cols], in0=z2_ps[:, :gw], scalar1=1.0,
                        scalar2=0.0, op0=mybir.AluOpType.mult,
                        op1=mybir.AluOpType.add,
                        accum_out=sumc2[:, gi:gi + 1])
                    nc.scalar.activation(
                        out=sq_s[:, :gw], in_=zT[:, cols],
                        func=mybir.ActivationFunctionType.Square,
                        accum_out=sumc2[:, ngrp + gi:ngrp + gi + 1])

                s2, d2 = _stats(nc, mp, dr, cfg, solo, sumc2, ngrp,
                                bn_t[:, 4 * l + 2:4 * l + 3],
                                bn_t[:, 4 * l + 3:4 * l + 4])

                # ---- h3 = relu(s2*z2+d2) -> hT; epilogue ----
                if l < L - 1:
                    ag_in = dr.tile([cfg.rps, F], dt.float32, tag="agin")
                    ag_out = dr.tile([cfg.tbl, F], dt.float32, tag="agout")
                    nc.sync.dma_start(out=ag_in[sh:sh + 1, :], in_=zrow[:])
                pooled_ps = ppool.tile([P, F], dt.float32, tag="poolps")
                for gi, grp in enumerate(cfg.groups):
                    gw = len(grp) * P
                    cols = slice(grp[0] * P, grp[0] * P + gw)
                    nc.scalar.activation(
                        out=hT[:, cols], in_=zT[:, cols],
                        func=mybir.ActivationFunctionType.Relu,
                        bias=d2[:, 0:1], scale=s2[:, 0:1])
                    if grp[-1] == nt - 1 and sp > sh:
                        nc.vector.memset(hT[:, sh:sp], 0.0)
                    stage = mp.tile([P, rt, P], dt.float32, tag="stage")
                    for t in grp:
                        ti = t - grp[0]
                        nm_ps = pp.tile([P, P], dt.float32, tag="nmps")
                        nc.tensor.transpose(
                            out=nm_ps[:], in_=hT[:, t * P:(t + 1) * P],
                            identity=ident[:])
                        nc.vector.tensor_copy(out=stage[:, ti, :], in_=nm_ps[:])
                        oh = mp.tile([P, P], dt.float32, tag="oh")
                        nc.vector.tensor_tensor(
                            out=oh[:], in0=gid_t[:, t:t + 1].to_broadcast([P, P]),
                            in1=iota_t[:], op=mybir.AluOpType.is_equal)
                        nc.tensor.matmul(
                            out=pooled_ps[:], lhsT=oh[:], rhs=stage[:, ti, :],
                            start=(t == 0), stop=(t == nt - 1),
                            skip_group_check=True)
                    jn = len(grp)
                    r0 = grp[0] * P
                    nc.sync.dma_start(
                        out=node_rep[r0:r0 + jn * P, l * F:(l + 1) * F]
                            .rearrange("(j p) f -> p j f", p=P),
                        in_=stage[:, :jn, :])
                    if l < L - 1:
                        rows = min(jn * P, sh - r0)
                        jf = rows // P
                        if jf > 0:
                            nc.sync.dma_start(
                                out=ag_in[r0:r0 + jf * P, :]
                                    .rearrange("(j p) f -> p j f", p=P),
                                in_=stage[:, :jf, :])
                        rem = rows - jf * P
                        if rem > 0:
                            nc.sync.dma_start(
                                out=ag_in[r0 + jf * P:r0 + jf * P + rem, :],
                                in_=stage[:rem, jf, :])
                pool_sb = mp.tile([P, F], dt.float32, tag="poolsb")
                nc.vector.tensor_copy(out=pool_sb[:], in_=pooled_ps[:])
                nc.sync.dma_start(out=pooled_o[l, :, :], in_=pool_sb[:])

                if l < L - 1:
                    if nocoll:
                        tables.append(x_pad.ap())
                    else:
                        nc.gpsimd.collective_compute(
                            "AllGather", mybir.AluOpType.bypass,
                            replica_groups=[list(range(NCORES))],
                            ins=[ag_in.opt()], outs=[ag_out.opt()])
                        tables.append(ag_out[:])

    nc.compile()
    return nc


def _stats(nc, mp, dr, cfg, solo, sumc, ngrp, gamma, beta):
    """Reduce per-group (sum, sumsq), AllReduce, return s=[g*rsqrt(var+eps)],
    d=[beta - mean*s] as [P,1] APs."""
    dt = mybir.dt
    st = mp.tile([P, 2], dt.float32, tag="st")
    nc.vector.tensor_reduce(
        out=st[:, 0:1], in_=sumc[:, 0:ngrp],
        axis=mybir.AxisListType.X, op=mybir.AluOpType.add)
    nc.vector.tensor_reduce(
        out=st[:, 1:2], in_=sumc[:, ngrp:2 * ngrp],
        axis=mybir.AxisListType.X, op=mybir.AluOpType.add)
    ar_i = dr.tile([P, 2], dt.float32, tag="ari")
    ar_o = dr.tile([P, 2], dt.float32, tag="aro")
    nc.gpsimd.dma_start(out=ar_i[:], in_=st[:])
    if solo or os.environ.get("KNOCOLL"):
        nc.gpsimd.dma_start(out=ar_o[:], in_=ar_i[:])
    else:
        nc.gpsimd.collective_compute(
            "AllReduce", mybir.AluOpType.add,
            replica_groups=[list(range(NCORES))],
            ins=[ar_i.opt()], outs=[ar_o.opt()])
    stg = mp.tile([P, 2], dt.float32, tag="stg")
    nc.sync.dma_start(out=stg[:], in_=ar_o[:])
    inv_n = 1.0 / cfg.n_nodes
    w = mp.tile([P, 6], dt.float32, tag="statw")
    # w: 0=mean 1=E2 2=var 3=recip->rsqrt 4=s 5=d
    nc.vector.tensor_scalar(out=w[:, 0:1], in0=stg[:, 0:1], scalar1=inv_n,
                            scalar2=None, op0=mybir.AluOpType.mult)
    nc.vector.tensor_scalar(out=w[:, 1:2], in0=stg[:, 1:2], scalar1=inv_n,
                            scalar2=None, op0=mybir.AluOpType.mult)
    nc.vector.tensor_tensor(out=w[:, 2:3], in0=w[:, 0:1], in1=w[:, 0:1],
                            op=mybir.AluOpType.mult)
    nc.vector.tensor_tensor(out=w[:, 2:3], in0=w[:, 1:2], in1=w[:, 2:3],
                            op=mybir.AluOpType.subtract)
    nc.vector.tensor_scalar(out=w[:, 2:3], in0=w[:, 2:3], scalar1=BN_EPS,
                            scalar2=None, op0=mybir.AluOpType.add)
    nc.vector.reciprocal(out=w[:, 3:4], in_=w[:, 2:3])
    nc.scalar.activation(out=w[:, 3:4], in_=w[:, 3:4],
                         func=mybir.ActivationFunctionType.Sqrt)
    nc.vector.tensor_tensor(out=w[:, 4:5], in0=gamma, in1=w[:, 3:4],
                            op=mybir.AluOpType.mult)
    nc.vector.tensor_tensor(out=w[:, 5:6], in0=w[:, 0:1], in1=w[:, 4:5],
                            op=mybir.AluOpType.mult)
    nc.vector.tensor_tensor(out=w[:, 5:6], in0=beta, in1=w[:, 5:6],
                            op=mybir.AluOpType.subtract)
    return w[:, 4:5], w[:, 5:6]


_CACHE = {}


def _get_nc(cfg, sched, key):
    if key not in _CACHE:
        _CACHE[key] = _build(cfg, sched)
    return _CACHE[key]


class _Exec:
    """PJRT executor mirroring bass2jax.run_bass_via_pjrt, with a cached
    jitted callable so executions can be repeated/timed."""

    def __init__(self, nc):
        import jax
        from jax.sharding import Mesh, PartitionSpec
        from jax.experimental.shard_map import shard_map
        from concourse import bass2jax, mybir as mb
        bass2jax.install_neuronx_cc_hook()
        self.jax = jax
        partition_name = (nc.partition_id_tensor.name
                          if nc.partition_id_tensor else None)
        in_names, out_names, out_avals, zero_outs = [], [], [], []
        for alloc in nc.m.functions[0].allocations:
            if not isinstance(alloc, mb.MemoryLocationSet):
                continue
            name = alloc.memorylocations[0].name
            if alloc.kind == "ExternalInput":
                if name != partition_name:
                    in_names.append(name)
            elif alloc.kind == "ExternalOutput":
                out_names.append(name)
                shape = tuple(alloc.tensor_shape)
                dtp = mb.dt.np(alloc.dtype)
                out_avals.append(jax.core.ShapedArray(shape, dtp))
                zero_outs.append(np.zeros(shape, dtp))
        self.in_names = list(in_names)
        self.out_names = out_names
        self.out_avals = out_avals
        self.zero_outs = zero_outs
        n_params, n_outs = len(in_names), len(out_avals)
        self.n_params = n_params
        all_in = in_names + out_names + ([partition_name] if partition_name else [])
        donate = tuple(range(n_params, n_params + n_outs))

        def _body(*args):
            operands = list(args)
            if partition_name is not None:
                operands.append(bass2jax.partition_id_tensor())
            outs = bass2jax._bass_exec_p.bind(
                *operands, out_avals=tuple(out_avals), in_names=tuple(all_in),
                out_names=tuple(out_names), lowering_input_output_aliases=(),
                sim_require_finite=True, sim_require_nnan=True, nc=nc)
            return tuple(outs)

        devices = jax.devices()[:NCORES]
        self.mesh = Mesh(np.asarray(devices), ("core",))
        self.spec = PartitionSpec("core")
        in_specs = (self.spec,) * (n_params + n_outs)
        out_specs = (self.spec,) * n_outs
        self.fn = jax.jit(
            shard_map(_body, mesh=self.mesh, in_specs=in_specs,
                      out_specs=out_specs, check_rep=False),
            donate_argnums=donate, keep_unused=True)
        self.dev_in = None

    def put_inputs(self, in_maps):
        import jax
        from jax.sharding import NamedSharding
        sh = NamedSharding(self.mesh, self.spec)
        concat = [np.concatenate([np.asarray(m[n]) for m in in_maps], axis=0)
                  for n in self.in_names]
        self.dev_in = [jax.device_put(a, sh) for a in concat]

    def _dev_zeros(self):
        import jax
        from jax.sharding import NamedSharding
        sh = NamedSharding(self.mesh, self.spec)
        return [jax.device_put(
            np.zeros((NCORES * z.shape[0], *z.shape[1:]), z.dtype), sh)
            for z in self.zero_outs]

    def execute(self):
        out = self.fn(*self.dev_in, *self._dev_zeros())
        self.jax.block_until_ready(out)
        return out

    def results(self, out):
        res = []
        for c in range(NCORES):
            res.append({
                name: np.asarray(out[i]).reshape(
                    NCORES, *self.out_avals[i].shape)[c]
                for i, name in enumerate(self.out_names)})
        return res

    def time(self, iters=5):
        import time
        ts = []
        for _ in range(iters):
            zeros = self._dev_zeros()
            self.jax.block_until_ready(zeros)
            t0 = time.perf_counter()
            out = self.fn(*self.dev_in, *zeros)
            self.jax.block_until_ready(out)
            ts.append(time.perf_counter() - t0)
        return ts


def _assemble(cfg, host, results):
    sh, n_g = cfg.shard, cfg.n_graphs
    node_rep = np.concatenate(
        [results[r]["node_rep"][:sh] for r in range(NCORES)], axis=0)
    graph_rep = np.zeros((n_g, L * F), dtype=np.float32)
    for r in range(NCORES):
        gb = int(host["gbase"][r])
        w = min(P, n_g - gb)
        blk = results[r]["pooled"]
        for l in range(L):
            graph_rep[gb:gb + w, l * F:(l + 1) * F] += blk[l][:w]
    return graph_rep, node_rep


def get_exec(cfg, inputs):
    in_maps, sched, host = _preprocess(cfg, **inputs)
    reps = os.environ.get("KREPS", "1")
    nc = _get_nc(cfg, sched, (cfg.n_nodes, cfg.n_edges, cfg.n_graphs, reps))
    key = ("exec", cfg.n_nodes, cfg.n_edges, cfg.n_graphs, reps)
    if key not in _CACHE:
        _CACHE[key] = _Exec(nc)
    ex = _CACHE[key]
    ex.put_inputs(in_maps)
    return ex, host


def run(cfg, inputs, trace=False):
    ex, host = get_exec(cfg, inputs)
    results = ex.results(ex.execute())
    graph_rep, node_rep = _assemble(cfg, host, results)

    class R:
        exec_time_ns = None
        instructions_and_trace = None
    r = R()
    r.results = results
    return (graph_rep, node_rep), r


def kernel(**inputs):
    cfg = Cfg()
    (graph_rep, node_rep), _ = run(cfg, inputs)
    return (graph_rep, node_rep)


# revision 18
# speedup vs baseline: 1.2449x; 1.0383x over previous
"""GIN encoder (3-layer, BN, scatter-add message passing) on 8 Trainium2 cores.

Strategy:
  - Nodes sharded contiguously across 8 cores (12500 each); edges owned by dst core.
  - Per layer: per-edge gather of h[src] rows via gpsimd.dma_gather from a
    replicated node-major table in DRAM (4 int16-addressable windows of 2 shards),
    segment-summed into feature-major agg^T tiles via PE matmuls with on-chip
    is_equal selection matrices; MLP runs feature-major with BN folded into
    per-column affines (b1/b2 cancel under BN); BN stats all-reduced across cores;
    next layer's table rebuilt via AllGather of per-core node-major shards.
  - Pooling via per-tile one-hot matmuls into a per-core graph window, combined
    on host; node_rep shards concatenated on host.
"""
import sys
sys.path.insert(0, "/opt/trn_rl_repo")
import math
import os
import numpy as np

import concourse.bass as bass
import concourse.bacc as bacc
import concourse.tile as tile
import concourse.mybir as mybir
from concourse.masks import make_identity

P = 128
F = 128            # feature dim (== P)
NCORES = 8
L = 3
BN_EPS = 1e-5
KT_CALL_CAP = 24   # max K-tiles per dma_gather call


class Cfg:
    def __init__(self, n_nodes=100000, n_edges=1600000, n_graphs=500, rt=4):
        assert n_nodes % NCORES == 0
        self.n_nodes = n_nodes
        self.n_edges = n_edges
        self.n_graphs = n_graphs
        self.shard = n_nodes // NCORES           # nodes per core
        self.nt = math.ceil(self.shard / P)      # dst tiles per core
        self.shard_pad = self.nt * P
        self.rps = self.shard + 1                # table rows per shard (+ zero row)
        self.tbl = self.rps * NCORES
        self.nwin = 4                            # int16 windows (2 shards each)
        self.win_rows = 2 * self.rps
        assert self.win_rows <= 32768
        self.rt = rt                             # dst tiles per gather group
        self.groups = [list(range(g, min(g + rt, self.nt)))
                       for g in range(0, self.nt, rt)]


def _preprocess(cfg, x, src, dst, batch, eps, W1, b1, g1, be1, W2, b2, gO, bO):
    n, sh, nt, nwin, rps = cfg.n_nodes, cfg.shard, cfg.nt, cfg.nwin, cfg.rps
    src = np.asarray(src, dtype=np.int64)
    dst = np.asarray(dst, dtype=np.int64)
    batch = np.asarray(batch, dtype=np.int64)
    x = np.asarray(x, dtype=np.float32)

    r_of = dst // sh
    dl = dst - r_of * sh
    t_of = dl >> 7
    dk = dl & 127
    c_of = src // (2 * sh)
    seg = ((r_of * nt + t_of) * nwin + c_of).astype(np.int64)
    okey = (seg << 7) | dk
    order = np.argsort(okey, kind="stable")
    counts = np.bincount(seg, minlength=NCORES * nt * nwin).reshape(NCORES, nt, nwin)
    seg_starts = np.zeros(NCORES * nt * nwin + 1, dtype=np.int64)
    np.cumsum(counts.reshape(-1), out=seg_starts[1:])

    # uniform K-tile counts across cores
    n_kt = np.ceil(counts.max(axis=0) / P).astype(np.int64)  # [nt, nwin]

    # window-local gather index for each edge's src
    src_r = src // sh
    src_win_local = (src_r - 2 * (src_r // 2)) * rps + (src - src_r * sh)
    assert src_win_local.max() < 2 * rps
    zero_idx = sh  # zero row of first shard in each window

    s_win = (n_kt.sum(axis=0) * P).astype(np.int64)   # slots per window
    kt_win = n_kt.sum(axis=0)                          # K-tiles per window

    idx_src_sorted = src_win_local[order].astype(np.int16)
    dkw_sorted = dk[order].astype(np.int16)

    # gather-call schedule (uniform across cores); per window, calls are
    # consumed in emission order, with "coff" the column offset into the
    # combined per-call stream [idx cols (ktn*8) | dstw cols (ktn)].
    calls = []
    kt_off = [0] * nwin    # running K-tile offset (for slot positions)
    co_off = [0] * nwin    # running column offset in comb arrays
    for grp in cfg.groups:
        gcalls = []
        for c in range(nwin):
            ktl = [(t, j) for t in grp for j in range(int(n_kt[t, c]))]
            ccalls = []
            for a in range(0, len(ktl), KT_CALL_CAP):
                chunk = ktl[a:a + KT_CALL_CAP]
                ccalls.append({"c": c, "kt": chunk, "off": kt_off[c],
                               "coff": co_off[c]})
                kt_off[c] += len(chunk)
                co_off[c] += len(chunk) * 9
            gcalls.append(ccalls)
        calls.append(gcalls)
    for c in range(nwin):
        assert kt_off[c] == kt_win[c]
    comb_cols = list(co_off)

    per_core = []
    for r in range(NCORES):
        streams = []
        for c in range(nwin):
            s_c = int(s_win[c])
            idx_s = np.full(s_c, zero_idx, dtype=np.int16)
            dstw_s = np.zeros(s_c, dtype=np.int16)
            off = 0
            for t in range(nt):
                cnt = int(counts[r, t, c])
                nkt = int(n_kt[t, c])
                if nkt == 0:
                    continue
                s0 = seg_starts[(r * nt + t) * nwin + c]
                idx_s[off:off + cnt] = idx_src_sorted[s0:s0 + cnt]
                dstw_s[off:off + cnt] = dkw_sorted[s0:s0 + cnt]
                off += nkt * P
            assert off == s_c
            # [128, S/16] replicated idx layout; [128, KT] dstw layout
            idx_a = np.tile(idx_s.reshape(-1, 16).T, (8, 1))
            dstw_a = dstw_s.reshape(-1, P).T
            streams.append((idx_a, dstw_a))
        combs = [np.zeros((P, comb_cols[c]), dtype=np.int16)
                 for c in range(nwin)]
        for gcalls in calls:
            for ccalls in gcalls:
                for cl in ccalls:
                    c = cl["c"]
                    ktn = len(cl["kt"])
                    o, co = cl["off"], cl["coff"]
                    idx_a, dstw_a = streams[c]
                    combs[c][:, co:co + ktn * 8] = idx_a[:, o * 8:(o + ktn) * 8]
                    combs[c][:, co + ktn * 8:co + ktn * 9] = dstw_a[:, o:o + ktn]
        per_core.append(combs)

    # graph windows
    gbase = np.array([batch[r * sh] for r in range(NCORES)], dtype=np.int64)
    gid_rel = np.full((NCORES, cfg.shard_pad), -1e6, dtype=np.float32)
    for r in range(NCORES):
        rel = (batch[r * sh:(r + 1) * sh] - gbase[r]).astype(np.float32)
        assert rel.max() < P, "graph span exceeds window"
        gid_rel[r, :sh] = rel
    gid_arrs = [gid_rel[r].reshape(-1, P).T.copy() for r in range(NCORES)]

    # padded node-major table for layer 0 (bf16)
    import ml_dtypes
    x_pad = np.zeros((cfg.tbl, F), dtype=ml_dtypes.bfloat16)
    for r in range(NCORES):
        x_pad[r * rps:r * rps + sh] = x[r * sh:(r + 1) * sh]

    # weights
    eps = np.asarray(eps, dtype=np.float32)
    wts = np.zeros((P, 9 * P), dtype=np.float32)
    for l in range(L):
        wts[:, (3 * l + 0) * P:(3 * l + 1) * P] = (1.0 + eps[l]) * W1[l]
        wts[:, (3 * l + 1) * P:(3 * l + 2) * P] = W1[l]
        wts[:, (3 * l + 2) * P:(3 * l + 3) * P] = W2[l]
    bn = np.zeros((P, 4 * L), dtype=np.float32)
    for l in range(L):
        bn[:, 4 * l + 0] = g1[l]
        bn[:, 4 * l + 1] = be1[l]
        bn[:, 4 * l + 2] = gO[l]
        bn[:, 4 * l + 3] = bO[l]
    iota = np.broadcast_to(np.arange(P, dtype=np.float32), (P, P)).copy()
    iota16 = iota.astype(ml_dtypes.bfloat16)

    in_maps = []
    for r in range(NCORES):
        xt = np.zeros((P, cfg.shard_pad), dtype=np.float32)
        xt[:, :sh] = x[r * sh:(r + 1) * sh].T
        m = {"x_pad": x_pad, "wts": wts, "bn": bn, "iota": iota,
             "iota16": iota16, "xT": xt, "gid": gid_arrs[r]}
        for c in range(nwin):
            m[f"comb_w{c}"] = per_core[r][c]
        in_maps.append(m)

    sched = {"n_kt": n_kt, "calls": calls, "comb_cols": comb_cols,
             "kt_win": kt_win}
    host = {"gbase": gbase}
    return in_maps, sched, host


def _build(cfg, sched):
    reps = int(os.environ.get("KREPS", "1"))
    solo = bool(os.environ.get("KSOLO"))
    nocoll = bool(os.environ.get("KNOCOLL")) or solo
    n_kt, calls = sched["n_kt"], sched["calls"]
    comb_cols = sched["comb_cols"]
    nt, nwin, sh = cfg.nt, cfg.nwin, cfg.shard
    sp, rt = cfg.shard_pad, cfg.rt
    ngrp = len(cfg.groups)
    dt = mybir.dt

    nc = bacc.Bacc("TRN2", target_bir_lowering=False, debug=False,
                   num_devices=1 if solo else NCORES)

    x_pad = nc.dram_tensor("x_pad", [cfg.tbl, F], dt.bfloat16, kind="ExternalInput")
    wts = nc.dram_tensor("wts", [P, 9 * P], dt.float32, kind="ExternalInput")
    bn = nc.dram_tensor("bn", [P, 4 * L], dt.float32, kind="ExternalInput")
    iota_d = nc.dram_tensor("iota", [P, P], dt.float32, kind="ExternalInput")
    iota16_d = nc.dram_tensor("iota16", [P, P], dt.bfloat16, kind="ExternalInput")
    xT_d = nc.dram_tensor("xT", [P, sp], dt.float32, kind="ExternalInput")
    gid_d = nc.dram_tensor("gid", [P, nt], dt.float32, kind="ExternalInput")
    comb_d = [nc.dram_tensor(f"comb_w{c}", [P, comb_cols[c]], dt.int16,
                             kind="ExternalInput") for c in range(nwin)]

    node_rep = nc.dram_tensor("node_rep", [sp, L * F], dt.float32,
                              kind="ExternalOutput")
    pooled_o = nc.dram_tensor("pooled", [L, P, F], dt.float32,
                              kind="ExternalOutput")

    with tile.TileContext(nc) as tc:
        with tc.tile_pool(name="big", bufs=1) as bigp, \
             tc.tile_pool(name="gat", bufs=2) as gp, \
             tc.tile_pool(name="mlp", bufs=3) as mp, \
             tc.tile_pool(name="cst", bufs=1) as cp, \
             tc.tile_pool(name="ps", bufs=2, space="PSUM") as pp, \
             tc.tile_pool(name="psp", bufs=1, space="PSUM") as ppool, \
             tc.tile_pool(name="dram", bufs=2, space="DRAM") as dr:

            wts_t = cp.tile([P, 9 * P], dt.float32, tag="wts")
            nc.sync.dma_start(out=wts_t[:], in_=wts[:])
            bn_t = cp.tile([P, 4 * L], dt.float32, tag="bn")
            nc.sync.dma_start(out=bn_t[:], in_=bn[:])
            iota_t = cp.tile([P, P], dt.float32, tag="iota")
            nc.sync.dma_start(out=iota_t[:], in_=iota_d[:])
            iota16_t = cp.tile([P, P], dt.bfloat16, tag="iota16")
            nc.sync.dma_start(out=iota16_t[:], in_=iota16_d[:])
            gid_t = cp.tile([P, nt], dt.float32, tag="gid")
            nc.sync.dma_start(out=gid_t[:], in_=gid_d[:])
            ident = cp.tile([P, P], dt.float32, tag="ident")
            make_identity(nc, ident[:])
            zrow = cp.tile([1, F], dt.bfloat16, tag="zrow")
            nc.vector.memset(zrow[:], 0.0)

            hT = bigp.tile([P, sp], dt.float32, tag="hT")
            zT = bigp.tile([P, sp], dt.float32, tag="zT")

            for _rep in range(reps):
              nc.sync.dma_start(out=hT[:], in_=xT_d[:])
              tables = [x_pad.ap()]
              for l in range(L):
                tbl_ap = tables[l]
                w1a = wts_t[:, (3 * l + 0) * P:(3 * l + 1) * P]
                w1 = wts_t[:, (3 * l + 1) * P:(3 * l + 2) * P]
                w2 = wts_t[:, (3 * l + 2) * P:(3 * l + 3) * P]

                sumc = mp.tile([P, 2 * ngrp], dt.float32, tag="sumc")
                # ---- aggregation + first linear ----
                for gi, grp in enumerate(cfg.groups):
                    gw = len(grp) * P
                    agg_ps = pp.tile([P, rt * P], dt.float32, tag="aggps")
                    nmm = sum(int(n_kt[t, :].sum()) for t in grp)
                    mmi = 0
                    for c in range(nwin):
                        for cl in calls[gi][c]:
                            ktn = len(cl["kt"])
                            slots = ktn * P
                            co = cl["coff"]
                            comb_t = gp.tile([P, ktn * 9], dt.int16, tag="comb")
                            nc.scalar.dma_start(
                                out=comb_t[:],
                                in_=comb_d[c][:, co:co + ktn * 9])
                            g_t = gp.tile([P, ktn, F], dt.bfloat16, tag="G")
                            nc.gpsimd.dma_gather(
                                out_ap=g_t[:],
                                in_ap=tbl_ap[c * cfg.win_rows:(c + 1) * cfg.win_rows, :],
                                idxs_ap=comb_t[:, :ktn * 8],
                                num_idxs=slots,
                                num_idxs_reg=slots,
                                elem_size=F,
                                single_packet=False,
                            )
                            dstw_f = gp.tile([P, ktn], dt.bfloat16, tag="dstwf")
                            nc.vector.tensor_copy(
                                out=dstw_f[:], in_=comb_t[:, ktn * 8:ktn * 9])
                            sel_t = gp.tile([P, ktn, P], dt.bfloat16, tag="sel")
                            nc.vector.tensor_tensor(
                                out=sel_t[:],
                                in0=dstw_f[:].to_broadcast([P, ktn, P]),
                                in1=iota16_t[:].rearrange("p (k e) -> p k e", k=1)
                                             .to_broadcast([P, ktn, P]),
                                op=mybir.AluOpType.is_equal)
                            for kk, (t, j) in enumerate(cl["kt"]):
                                ti = t - grp[0]
                                nc.tensor.matmul(
                                    out=agg_ps[:, ti * P:(ti + 1) * P],
                                    lhsT=g_t[:, kk, :],
                                    rhs=sel_t[:, kk, :],
                                    start=(mmi == 0),
                                    stop=(mmi == nmm - 1),
                                    skip_group_check=True)
                                mmi += 1
                    cols = slice(grp[0] * P, grp[0] * P + gw)
                    agg_sb = mp.tile([P, rt * P], dt.float32, tag="aggsb")
                    nc.vector.tensor_copy(out=agg_sb[:, :gw], in_=agg_ps[:, :gw])
                    for t in grp:
                        if int(n_kt[t, :].sum()) == 0:
                            ti = t - grp[0]
                            nc.vector.memset(agg_sb[:, ti * P:(ti + 1) * P], 0.0)
                    z_ps = pp.tile([P, rt * P], dt.float32, tag="zps")
                    nc.tensor.matmul(out=z_ps[:, :gw], lhsT=w1a, rhs=hT[:, cols],
                                     start=True, stop=False, skip_group_check=True)
                    nc.tensor.matmul(out=z_ps[:, :gw], lhsT=w1, rhs=agg_sb[:, :gw],
                                     start=False, stop=True, skip_group_check=True)
                    sq_s = mp.tile([P, rt * P], dt.float32, tag="sqs")
                    nc.vector.tensor_scalar(
                        out=zT[:, cols], in0=z_ps[:, :gw], scalar1=1.0, scalar2=0.0,
                        op0=mybir.AluOpType.mult, op1=mybir.AluOpType.add,
                        accum_out=sumc[:, gi:gi + 1])
                    nc.scalar.activation(
                        out=sq_s[:, :gw], in_=zT[:, cols],
                        func=mybir.ActivationFunctionType.Square,
                        accum_out=sumc[:, ngrp + gi:ngrp + gi + 1])

                # ---- BN1 ----
                s1, d1 = _stats(nc, mp, dr, cfg, solo, sumc, ngrp,
                                bn_t[:, 4 * l:4 * l + 1],
                                bn_t[:, 4 * l + 1:4 * l + 2])

                # ---- t = relu(s1*z+d1); z2 = W2^T t; stats2 ----
                sumc2 = mp.tile([P, 2 * ngrp], dt.float32, tag="sumc2")
                for gi, grp in enumerate(cfg.groups):
                    gw = len(grp) * P
                    cols = slice(grp[0] * P, grp[0] * P + gw)
                    t_s = mp.tile([P, rt * P], dt.float32, tag="ts")
                    nc.scalar.activation(
                        out=t_s[:, :gw], in_=zT[:, cols],
                        func=mybir.ActivationFunctionType.Relu,
                        bias=d1[:, 0:1], scale=s1[:, 0:1])
                    if grp[-1] == nt - 1 and sp > sh:
                        po = sh - grp[0] * P
                        nc.vector.memset(t_s[:, po:gw], 0.0)
                    z2_ps = pp.tile([P, rt * P], dt.float32, tag="zps")
                    nc.tensor.matmul(out=z2_ps[:, :gw], lhsT=w2, rhs=t_s[:, :gw],
                                     start=True, stop=True, skip_group_check=True)
                    sq_s = mp.tile([P, rt * P], dt.float32, tag="sqs")
                    nc.vector.tensor_scalar(
                        out=zT[:, cols], in0=z2_ps[:, :gw], scalar1=1.0,
                        scalar2=0.0, op0=mybir.AluOpType.mult,
                        op1=mybir.AluOpType.add,
                        accum_out=sumc2[:, gi:gi + 1])
                    nc.scalar.activation(
                        out=sq_s[:, :gw], in_=zT[:, cols],
                        func=mybir.ActivationFunctionType.Square,
                        accum_out=sumc2[:, ngrp + gi:ngrp + gi + 1])

                s2, d2 = _stats(nc, mp, dr, cfg, solo, sumc2, ngrp,
                                bn_t[:, 4 * l + 2:4 * l + 3],
                                bn_t[:, 4 * l + 3:4 * l + 4])

                # ---- h3 = relu(s2*z2+d2) -> hT; epilogue ----
                if l < L - 1:
                    ag_in = dr.tile([cfg.rps, F], dt.bfloat16, tag="agin")
                    ag_out = dr.tile([cfg.tbl, F], dt.bfloat16, tag="agout")
                    nc.sync.dma_start(out=ag_in[sh:sh + 1, :], in_=zrow[:])
                pooled_ps = ppool.tile([P, F], dt.float32, tag="poolps")
                for gi, grp in enumerate(cfg.groups):
                    gw = len(grp) * P
                    cols = slice(grp[0] * P, grp[0] * P + gw)
                    nc.scalar.activation(
                        out=hT[:, cols], in_=zT[:, cols],
                        func=mybir.ActivationFunctionType.Relu,
                        bias=d2[:, 0:1], scale=s2[:, 0:1])
                    if grp[-1] == nt - 1 and sp > sh:
                        nc.vector.memset(hT[:, sh:sp], 0.0)
                    stage = mp.tile([P, rt, P], dt.float32, tag="stage")
                    for t in grp:
                        ti = t - grp[0]
                        nm_ps = pp.tile([P, P], dt.float32, tag="nmps")
                        nc.tensor.transpose(
                            out=nm_ps[:], in_=hT[:, t * P:(t + 1) * P],
                            identity=ident[:])
                        nc.vector.tensor_copy(out=stage[:, ti, :], in_=nm_ps[:])
                        oh = mp.tile([P, P], dt.float32, tag="oh")
                        nc.vector.tensor_tensor(
                            out=oh[:], in0=gid_t[:, t:t + 1].to_broadcast([P, P]),
                            in1=iota_t[:], op=mybir.AluOpType.is_equal)
                        nc.tensor.matmul(
                            out=pooled_ps[:], lhsT=oh[:], rhs=stage[:, ti, :],
                            start=(t == 0), stop=(t == nt - 1),
                            skip_group_check=True)
                    jn = len(grp)
                    r0 = grp[0] * P
                    nc.sync.dma_start(
                        out=node_rep[r0:r0 + jn * P, l * F:(l + 1) * F]
                            .rearrange("(j p) f -> p j f", p=P),
                        in_=stage[:, :jn, :])
                    if l < L - 1:
                        rows = min(jn * P, sh - r0)
                        jf = rows // P
                        if jf > 0:
                            nc.gpsimd.dma_start(
                                out=ag_in[r0:r0 + jf * P, :]
                                    .rearrange("(j p) f -> p j f", p=P),
                                in_=stage[:, :jf, :])
                        rem = rows - jf * P
                        if rem > 0:
                            nc.gpsimd.dma_start(
                                out=ag_in[r0 + jf * P:r0 + jf * P + rem, :],
                                in_=stage[:rem, jf, :])
                pool_sb = mp.tile([P, F], dt.float32, tag="poolsb")
                nc.vector.tensor_copy(out=pool_sb[:], in_=pooled_ps[:])
                nc.sync.dma_start(out=pooled_o[l, :, :], in_=pool_sb[:])

                if l < L - 1:
                    if nocoll:
                        tables.append(x_pad.ap())
                    else:
                        nc.gpsimd.collective_compute(
                            "AllGather", mybir.AluOpType.bypass,
                            replica_groups=[list(range(NCORES))],
                            ins=[ag_in.opt()], outs=[ag_out.opt()])
                        tables.append(ag_out[:])

    nc.compile()
    return nc


def _stats(nc, mp, dr, cfg, solo, sumc, ngrp, gamma, beta):
    """Reduce per-group (sum, sumsq), AllReduce, return s=[g*rsqrt(var+eps)],
    d=[beta - mean*s] as [P,1] APs."""
    dt = mybir.dt
    st = mp.tile([P, 2], dt.float32, tag="st")
    nc.vector.tensor_reduce(
        out=st[:, 0:1], in_=sumc[:, 0:ngrp],
        axis=mybir.AxisListType.X, op=mybir.AluOpType.add)
    nc.vector.tensor_reduce(
        out=st[:, 1:2], in_=sumc[:, ngrp:2 * ngrp],
        axis=mybir.AxisListType.X, op=mybir.AluOpType.add)
    ar_i = dr.tile([P, 2], dt.float32, tag="ari")
    ar_o = dr.tile([P, 2], dt.float32, tag="aro")
    nc.gpsimd.dma_start(out=ar_i[:], in_=st[:])
    if solo or os.environ.get("KNOCOLL"):
        nc.gpsimd.dma_start(out=ar_o[:], in_=ar_i[:])
    else:
        nc.gpsimd.collective_compute(
            "AllReduce", mybir.AluOpType.add,
            replica_groups=[list(range(NCORES))],
            ins=[ar_i.opt()], outs=[ar_o.opt()])
    stg = mp.tile([P, 2], dt.float32, tag="stg")
    nc.sync.dma_start(out=stg[:], in_=ar_o[:])
    inv_n = 1.0 / cfg.n_nodes
    w = mp.tile([P, 6], dt.float32, tag="statw")
    # w: 0=mean 1=E2 2=var 3=recip->rsqrt 4=s 5=d
    nc.vector.tensor_scalar(out=w[:, 0:1], in0=stg[:, 0:1], scalar1=inv_n,
                            scalar2=None, op0=mybir.AluOpType.mult)
    nc.vector.tensor_scalar(out=w[:, 1:2], in0=stg[:, 1:2], scalar1=inv_n,
                            scalar2=None, op0=mybir.AluOpType.mult)
    nc.vector.tensor_tensor(out=w[:, 2:3], in0=w[:, 0:1], in1=w[:, 0:1],
                            op=mybir.AluOpType.mult)
    nc.vector.tensor_tensor(out=w[:, 2:3], in0=w[:, 1:2], in1=w[:, 2:3],
                            op=mybir.AluOpType.subtract)
    nc.vector.tensor_scalar(out=w[:, 2:3], in0=w[:, 2:3], scalar1=BN_EPS,
                            scalar2=None, op0=mybir.AluOpType.add)
    nc.vector.reciprocal(out=w[:, 3:4], in_=w[:, 2:3])
    nc.scalar.activation(out=w[:, 3:4], in_=w[:, 3:4],
                         func=mybir.ActivationFunctionType.Sqrt)
    nc.vector.tensor_tensor(out=w[:, 4:5], in0=gamma, in1=w[:, 3:4],
                            op=mybir.AluOpType.mult)
    nc.vector.tensor_tensor(out=w[:, 5:6], in0=w[:, 0:1], in1=w[:, 4:5],
                            op=mybir.AluOpType.mult)
    nc.vector.tensor_tensor(out=w[:, 5:6], in0=beta, in1=w[:, 5:6],
                            op=mybir.AluOpType.subtract)
    return w[:, 4:5], w[:, 5:6]


_CACHE = {}


def _get_nc(cfg, sched, key):
    if key not in _CACHE:
        _CACHE[key] = _build(cfg, sched)
    return _CACHE[key]


class _Exec:
    """PJRT executor mirroring bass2jax.run_bass_via_pjrt, with a cached
    jitted callable so executions can be repeated/timed."""

    def __init__(self, nc):
        import jax
        from jax.sharding import Mesh, PartitionSpec
        from jax.experimental.shard_map import shard_map
        from concourse import bass2jax, mybir as mb
        bass2jax.install_neuronx_cc_hook()
        self.jax = jax
        partition_name = (nc.partition_id_tensor.name
                          if nc.partition_id_tensor else None)
        in_names, out_names, out_avals, zero_outs = [], [], [], []
        for alloc in nc.m.functions[0].allocations:
            if not isinstance(alloc, mb.MemoryLocationSet):
                continue
            name = alloc.memorylocations[0].name
            if alloc.kind == "ExternalInput":
                if name != partition_name:
                    in_names.append(name)
            elif alloc.kind == "ExternalOutput":
                out_names.append(name)
                shape = tuple(alloc.tensor_shape)
                dtp = mb.dt.np(alloc.dtype)
                out_avals.append(jax.core.ShapedArray(shape, dtp))
                zero_outs.append(np.zeros(shape, dtp))
        self.in_names = list(in_names)
        self.out_names = out_names
        self.out_avals = out_avals
        self.zero_outs = zero_outs
        n_params, n_outs = len(in_names), len(out_avals)
        self.n_params = n_params
        all_in = in_names + out_names + ([partition_name] if partition_name else [])
        donate = tuple(range(n_params, n_params + n_outs))

        def _body(*args):
            operands = list(args)
            if partition_name is not None:
                operands.append(bass2jax.partition_id_tensor())
            outs = bass2jax._bass_exec_p.bind(
                *operands, out_avals=tuple(out_avals), in_names=tuple(all_in),
                out_names=tuple(out_names), lowering_input_output_aliases=(),
                sim_require_finite=True, sim_require_nnan=True, nc=nc)
            return tuple(outs)

        devices = jax.devices()[:NCORES]
        self.mesh = Mesh(np.asarray(devices), ("core",))
        self.spec = PartitionSpec("core")
        in_specs = (self.spec,) * (n_params + n_outs)
        out_specs = (self.spec,) * n_outs
        self.fn = jax.jit(
            shard_map(_body, mesh=self.mesh, in_specs=in_specs,
                      out_specs=out_specs, check_rep=False),
            donate_argnums=donate, keep_unused=True)
        self.dev_in = None

    def put_inputs(self, in_maps):
        import jax
        from jax.sharding import NamedSharding
        sh = NamedSharding(self.mesh, self.spec)
        concat = [np.concatenate([np.asarray(m[n]) for m in in_maps], axis=0)
                  for n in self.in_names]
        self.dev_in = [jax.device_put(a, sh) for a in concat]

    def _dev_zeros(self):
        import jax
        from jax.sharding import NamedSharding
        sh = NamedSharding(self.mesh, self.spec)
        return [jax.device_put(
            np.zeros((NCORES * z.shape[0], *z.shape[1:]), z.dtype), sh)
            for z in self.zero_outs]

    def execute(self):
        out = self.fn(*self.dev_in, *self._dev_zeros())
        self.jax.block_until_ready(out)
        return out

    def results(self, out):
        res = []
        for c in range(NCORES):
            res.append({
                name: np.asarray(out[i]).reshape(
                    NCORES, *self.out_avals[i].shape)[c]
                for i, name in enumerate(self.out_names)})
        return res

    def time(self, iters=5):
        import time
        ts = []
        for _ in range(iters):
            zeros = self._dev_zeros()
            self.jax.block_until_ready(zeros)
            t0 = time.perf_counter()
            out = self.fn(*self.dev_in, *zeros)
            self.jax.block_until_ready(out)
            ts.append(time.perf_counter() - t0)
        return ts


def _assemble(cfg, host, results):
    sh, n_g = cfg.shard, cfg.n_graphs
    node_rep = np.concatenate(
        [results[r]["node_rep"][:sh] for r in range(NCORES)], axis=0)
    graph_rep = np.zeros((n_g, L * F), dtype=np.float32)
    for r in range(NCORES):
        gb = int(host["gbase"][r])
        w = min(P, n_g - gb)
        blk = results[r]["pooled"]
        for l in range(L):
            graph_rep[gb:gb + w, l * F:(l + 1) * F] += blk[l][:w]
    return graph_rep, node_rep


def get_exec(cfg, inputs):
    in_maps, sched, host = _preprocess(cfg, **inputs)
    reps = os.environ.get("KREPS", "1")
    nc = _get_nc(cfg, sched, (cfg.n_nodes, cfg.n_edges, cfg.n_graphs, reps))
    key = ("exec", cfg.n_nodes, cfg.n_edges, cfg.n_graphs, reps)
    if key not in _CACHE:
        _CACHE[key] = _Exec(nc)
    ex = _CACHE[key]
    ex.put_inputs(in_maps)
    return ex, host


def run(cfg, inputs, trace=False):
    ex, host = get_exec(cfg, inputs)
    results = ex.results(ex.execute())
    graph_rep, node_rep = _assemble(cfg, host, results)

    class R:
        exec_time_ns = None
        instructions_and_trace = None
    r = R()
    r.results = results
    return (graph_rep, node_rep), r


def kernel(**inputs):
    cfg = Cfg()
    (graph_rep, node_rep), _ = run(cfg, inputs)
    return (graph_rep, node_rep)
